# revision 27
# baseline (speedup 1.0000x reference)
"""AttentionBlock (GroupNorm + 4-head self-attention + proj + residual) on 8 trn2 cores.

Sharding: data-parallel over batch (B=16 -> 2 per core). Each core runs the full
block on its 2 batch elements; no collectives.

Device pipeline per batch (all layouts chosen so no on-device transposes are needed):
  - GroupNorm stats via bn_stats/bn_aggr + tiny PE matmuls for the cross-partition
    group combine (gamma/beta folded into the qkv weights on host).
  - Q,K GEMM in [channel, spatial] layout; V^T via a swapped GEMM (stationary=h).
  - Scores computed transposed: S^T[m,n] = K^T Q, so softmax's sum runs over the
    PSUM partition axis, computed by an all-ones stationary matmul that also
    replicates the denominator Z across partitions (no broadcast needed).
  - exp on ACT straight out of PSUM; no max-subtraction (scores bounded ~|8|).
  - AV as O^T[d,n] = sum_m V^T[m,d] expS^T[m,n], 2 heads packed per 128-col array.
  - proj GEMM + residual; biases injected via rank-1 (K=1) matmuls into PSUM.
"""

import numpy as np
from contextlib import ExitStack

import concourse.bass as bass
import concourse.bacc as bacc
import concourse.tile as tile
import concourse.mybir as mybir
from concourse.bass_utils import run_bass_kernel_spmd

F32 = mybir.dt.float32
F32R = mybir.dt.float32r
BF16 = mybir.dt.bfloat16

B, C, HH, WW = 16, 256, 32, 32
N = HH * WW           # 1024 spatial positions
NH = 4                # heads
D = C // NH           # 64 head dim
G = 32                # groups
EPS = 1e-5
NCORES = 8
BL = B // NCORES      # batches per core

USE_F32R = True       # fp32r matmuls: full PE rate; flip to False for exact fp32
MM_DT = F32R if USE_F32R else F32


def _R(ap):
    return ap


def build_bass():
    nc = bacc.Bacc("TRN2", target_bir_lowering=False, debug=False)

    x_d = nc.dram_tensor("x", [BL, C, N], F32, kind="ExternalInput").ap()
    wqk_d = nc.dram_tensor("wqk_t", [2, 128, 512], MM_DT, kind="ExternalInput").ap()
    wv_d = nc.dram_tensor("wv_t", [2, 128, 256], MM_DT, kind="ExternalInput").ap()
    wp_d = nc.dram_tensor("wp_t", [2, 128, 256], MM_DT, kind="ExternalInput").ap()
    bqk_d = nc.dram_tensor("bqk", [4, 128], F32, kind="ExternalInput").ap()
    bv_d = nc.dram_tensor("bv", [1, 256], MM_DT, kind="ExternalInput").ap()
    bp_d = nc.dram_tensor("bp", [1, 256], MM_DT, kind="ExternalInput").ap()
    gmap_d = nc.dram_tensor("gmap", [128, 16], F32, kind="ExternalInput").ap()
    gexp_d = nc.dram_tensor("gexp", [16, 128], F32, kind="ExternalInput").ap()
    y_d = nc.dram_tensor("y", [BL, C, N], F32, kind="ExternalOutput").ap()

    Exp = mybir.ActivationFunctionType.Exp
    mult = mybir.AluOpType.mult
    sub = mybir.AluOpType.subtract
    add = mybir.AluOpType.add

    with tile.TileContext(nc) as tc, ExitStack() as ctx:
        consts = ctx.enter_context(tc.tile_pool(name="consts", bufs=1))
        xpool = ctx.enter_context(tc.tile_pool(name="xp", bufs=1))
        hpool = ctx.enter_context(tc.tile_pool(name="hp", bufs=1))
        qkpool = ctx.enter_context(tc.tile_pool(name="qkp", bufs=1))
        vtpool = ctx.enter_context(tc.tile_pool(name="vtp", bufs=1))
        opool = ctx.enter_context(tc.tile_pool(name="op", bufs=1))
        gnpool = ctx.enter_context(tc.tile_pool(name="gnp", bufs=2))
        expool = ctx.enter_context(tc.tile_pool(name="exp", bufs=4))
        rzpool = ctx.enter_context(tc.tile_pool(name="rzp", bufs=3))
        outpool = ctx.enter_context(tc.tile_pool(name="outp", bufs=2))
        ps_big = ctx.enter_context(tc.tile_pool(name="psb", bufs=3, space="PSUM"))
        ps_o = ctx.enter_context(tc.tile_pool(name="pso", bufs=2, space="PSUM"))
        
        x_sb = [[None] * 2 for _ in range(BL)]
        h_sb = [[None] * 2 for _ in range(BL)]
        qk_sb = [[None] * 4 for _ in range(BL)]
        vt_sb = [[None] * 8 for _ in range(BL)]
        o_sb = [[None] * 2 for _ in range(BL)]

        # --- x first (GroupNorm gates everything), split for earlier stats ---
        for b in range(BL):
            for ct in range(2):
                xt = xpool.tile([128, N], F32, tag=f"x{b}{ct}", name=f"x{b}{ct}")
                x_sb[b][ct] = xt
                nc.sync.dma_start(xt[:, 0:512], x_d[b, ct * 128:(ct + 1) * 128, 0:512])
                nc.sync.dma_start(xt[:, 512:1024], x_d[b, ct * 128:(ct + 1) * 128, 512:1024])

        # --- constants / weights into SBUF ---
        wqk_sb = [consts.tile([128, 512], MM_DT, tag=f"wqk{k}", name=f"wqk{k}") for k in range(2)]
        wv_sb = [consts.tile([128, 256], MM_DT, tag=f"wv{k}", name=f"wv{k}") for k in range(2)]
        wp_sb = [consts.tile([128, 256], MM_DT, tag=f"wp{k}", name=f"wp{k}") for k in range(2)]
        gmap_sb = consts.tile([128, 16], F32, tag="gmap")
        nc.sync.dma_start(gmap_sb[:], gmap_d[:])
        gexp_sb = consts.tile([16, 128], F32, tag="gexp")
        nc.sync.dma_start(gexp_sb[:], gexp_d[:])
        bqk_sb = consts.tile([128, 4], F32, tag="bqk")
        nc.sync.dma_start(bqk_sb[:], bqk_d.transpose([1, 0]))
        for k in range(2):
            nc.sync.dma_start(wqk_sb[k][:], wqk_d[k])
            nc.sync.dma_start(wv_sb[k][:], wv_d[k])
            nc.sync.dma_start(wp_sb[k][:], wp_d[k])
        bv_sb = consts.tile([1, 256], MM_DT, tag="bv")
        nc.sync.dma_start(bv_sb[:], bv_d[:])
        bp_sb = consts.tile([1, 256], MM_DT, tag="bp")
        nc.sync.dma_start(bp_sb[:], bp_d[:])
        ones_f32 = consts.tile([128, 512], F32, tag="ones_f32")
        nc.vector.memset(ones_f32[:], 1.0)
        ones_sb = consts.tile([128, 512], MM_DT, tag="ones")
        nc.vector.tensor_copy(ones_sb[:], ones_f32[:])
        # bv broadcast to all partitions via a rank-1 matmul (done once)
        pbv = ps_o.tile([128, 256], F32, tag="o", name="pbv")
        nc.tensor.matmul(pbv[:], lhsT=_R(ones_sb[0:1, 0:128]), rhs=_R(bv_sb[0:1, :]),
                         start=True, stop=True)
        bvb = consts.tile([128, 256], F32, tag="bvb")
        nc.vector.tensor_copy(bvb[:], pbv[:])

        # ====== Phase A: GroupNorm, batched 4-wide across (b, ct) tiles ======
        # Stats per tile, then ONE short scalar-math chain on [128, 4] strided
        # views instead of four serial per-tile chains.
        units4 = [(b, ct) for b in range(BL) for ct in range(2)]
        bn6 = gnpool.tile([128, 48], F32, tag="bn6")
        mva = gnpool.tile([128, 8], F32, tag="mva")  # (mean, var) x 4 units
        for u4, (b, ct) in enumerate(units4):
            xt = x_sb[b][ct]
            nc.vector.bn_stats(bn6[:, 12 * u4:12 * u4 + 6], xt[:, 0:512])
            nc.vector.bn_stats(bn6[:, 12 * u4 + 6:12 * u4 + 12], xt[:, 512:1024])
            nc.vector.bn_aggr(mva[:, 2 * u4:2 * u4 + 2], bn6[:, 12 * u4:12 * u4 + 12])
        mva2 = mva[:].rearrange("p (u c) -> p u c", c=2)
        m2a = gnpool.tile([128, 4], F32, tag="m2a")
        nc.vector.tensor_mul(m2a[:], mva2[:, :, 0], mva2[:, :, 0])
        # group-combine via matmul: [sum mean, sum var]x4 then [sum mean^2]x4
        psg = ps_o.tile([16, 12], F32, tag="o", name="psg")
        nc.tensor.matmul(psg[:, 0:8], lhsT=gmap_sb[:], rhs=mva[:], start=True, stop=True,
                         skip_group_check=True)
        nc.tensor.matmul(psg[:, 8:12], lhsT=gmap_sb[:], rhs=m2a[:], start=True, stop=True,
                         skip_group_check=True)
        gsa = gnpool.tile([16, 12], F32, tag="gsa")
        nc.vector.tensor_copy(gsa[:], psg[:])
        gsa2 = gsa[:, 0:8].rearrange("p (u c) -> p u c", c=2)
        nc.vector.tensor_add(gsa2[:, :, 1], gsa2[:, :, 1], gsa[:, 8:12])
        psc = ps_o.tile([128, 8], F32, tag="o", name="psc")
        nc.tensor.matmul(psc[:], lhsT=gexp_sb[:], rhs=gsa[:, 0:8], start=True, stop=True)
        csa = gnpool.tile([128, 8], F32, tag="csa")  # [mean_g, E[x^2]_g] x 4
        nc.vector.tensor_copy(csa[:], psc[:])
        csa2 = csa[:].rearrange("p (u c) -> p u c", c=2)
        m2g = gnpool.tile([128, 4], F32, tag="m2g")
        nc.vector.tensor_mul(m2g[:], csa2[:, :, 0], csa2[:, :, 0])
        vea = gnpool.tile([128, 4], F32, tag="vea")
        nc.vector.tensor_sub(vea[:], csa2[:, :, 1], m2g[:])
        nc.vector.tensor_scalar_add(vea[:], vea[:], EPS)
        iva = gnpool.tile([128, 4], F32, tag="iva")
        nc.vector.reciprocal(iva[:], vea[:])
        s_a = gnpool.tile([128, 4], F32, tag="s_a")
        nc.scalar.sqrt(s_a[:], iva[:])
        t_a = gnpool.tile([128, 4], F32, tag="t_a")  # mean * inv_std
        nc.vector.tensor_mul(t_a[:], csa2[:, :, 0], s_a[:])
        # PE warm-up: HAM needs ~3.4us of sustained activity; these rewrite
        # pbv with the same value while the DVE finishes the GN chain.
        for w in range(14):
            nc.tensor.matmul(pbv[:], lhsT=_R(ones_sb[0:1, 0:128]),
                             rhs=_R(bv_sb[0:1, :]), start=True, stop=True)
        for u4, (b, ct) in enumerate(units4):
            # h = x * s - t  (gamma/beta already folded into W/b on host)
            ht = hpool.tile([128, N], MM_DT, tag=f"h{b}{ct}", name=f"h{b}{ct}")
            h_sb[b][ct] = ht
            nc.vector.tensor_scalar(ht[:], x_sb[b][ct][:], s_a[:, u4:u4 + 1],
                                    t_a[:, u4:u4 + 1], mult, sub)

        def emit_qkgemm(b):
            # Q,K GEMM: out channels ot: 0=q01 1=q23 2=k01 3=k23 (K pre-scaled by 1/8)
            for ot in range(4):
                pq = ps_big.tile([128, N], F32, tag="big")
                for nch in range(2):
                    ns = slice(nch * 512, (nch + 1) * 512)
                    for k in range(2):
                        nc.tensor.matmul(
                            pq[:, ns],
                            lhsT=_R(wqk_sb[k][:, ot * 128:(ot + 1) * 128]),
                            rhs=_R(h_sb[b][k][:, ns]),
                            start=(k == 0), stop=(k == 1),
                        )
                qk = qkpool.tile([128, N], MM_DT, tag=f"qk{b}{ot}")
                qk_sb[b][ot] = qk
                nc.vector.tensor_scalar(qk[:], pq[:], bqk_sb[:, ot:ot + 1], None, add)

        def emit_vgemm(b):
            # V^T GEMM: V^T[m, vc] = sum_c h[c,m] WvT[c,vc]
            for m in range(8):
                pv = ps_o.tile([128, 512], F32, tag="o")
                mc = slice(m * 128, (m + 1) * 128)
                for k in range(2):
                    nc.tensor.matmul(
                        pv[:, 0:256],
                        lhsT=_R(h_sb[b][k][:, mc]),
                        rhs=_R(wv_sb[k][:]),
                        start=(k == 0), stop=(k == 1),
                    )
                # vt layout [128, 512]: per global head h a 128-col block;
                # even h: [V_h | ones], odd h: [ones | V_h].  The ones columns
                # make the AV matmul also emit the softmax denominator Z
                # (replicated over 64 partitions) in the same PSUM bank.
                vt = vtpool.tile([128, 512], BF16, tag=f"vt{b}{m}")
                vt_sb[b][m] = vt
                vt4 = vt[:].rearrange("p (a u v d) -> p a u v d", a=2, u=2, v=2)
                pv4 = pv[:, 0:256].rearrange("p (a w d) -> p a w d", a=2, w=2)
                of = ones_f32[:, 0:128].rearrange("p (a d) -> p a d", a=2)
                bvb4 = bvb[:].rearrange("p (a w d) -> p a w d", a=2, w=2)
                nc.vector.tensor_add(vt4[:, :, 0, 0, :], pv4[:, :, 0, :], bvb4[:, :, 0, :])
                nc.vector.tensor_add(vt4[:, :, 1, 1, :], pv4[:, :, 1, :], bvb4[:, :, 1, :])
                vtq = vt[:].rearrange("p (a q d) -> p a q d", a=2, q=4)
                nc.vector.tensor_copy(vtq[:, :, 1:3, :], ones_f32[:, 0:256].rearrange(
                    "p (a d) -> p a d", a=2).rearrange("p a (u d) -> p a u d", u=2))

        emit_qkgemm(0)
        emit_vgemm(0)
        emit_qkgemm(1)

        # ================= Phase B: attention, Phase C: proj+residual ======
        # Software-pipelined emission over all (b, p, nch) units x 8 m-chunks:
        # the PE executes its queue in order, so S^T matmuls are emitted two
        # steps ahead of the exp-dependent AV+Z matmuls to keep the PE from
        # stalling on ACT.
        for b in range(BL):
            for p in range(2):
                o_sb[b][p] = opool.tile([128, N], MM_DT, tag=f"o{b}{p}",
                                        name=f"ot{b}{p}")
        units = [(b, p, nch) for b in range(BL) for p in range(2) for nch in range(2)]
        seq = [(u, m) for u in range(len(units)) for m in range(8)]
        s_tiles = {}
        po_tiles = {}

        def emit_S(i):
            u, m = seq[i]
            b, p, nch = units[u]
            qt, kt = qk_sb[b][p], qk_sb[b][2 + p]
            ns = slice(nch * 512, (nch + 1) * 512)
            mc = slice(m * 128, (m + 1) * 128)
            ps = ps_big.tile([128, N], F32, tag="big", name="ps")
            nc.tensor.matmul(ps[:, 0:512], lhsT=_R(kt[0:64, mc]),
                             rhs=_R(qt[0:64, ns]), start=True, stop=True)
            nc.tensor.matmul(ps[:, 512:1024], lhsT=_R(kt[64:128, mc]),
                             rhs=_R(qt[64:128, ns]), start=True, stop=True)
            s_tiles[i] = ps

        def emit_proj(b):
            # proj + residual + bias
            for ct in range(2):
                pp = ps_big.tile([128, N], F32, tag="big", name="pp")
                for nch in range(2):
                    ns = slice(nch * 512, (nch + 1) * 512)
                    for k in range(2):
                        nc.tensor.matmul(
                            pp[:, ns],
                            lhsT=_R(wp_sb[k][:, ct * 128:(ct + 1) * 128]),
                            rhs=_R(o_sb[b][k][:, ns]),
                            start=(k == 0), stop=False,
                        )
                    nc.tensor.matmul(
                        pp[:, ns],
                        lhsT=_R(bp_sb[0:1, ct * 128:(ct + 1) * 128]),
                        rhs=_R(ones_sb[0:1, :]),
                        start=False, stop=True,
                    )
                outt = outpool.tile([128, N], F32, tag="out")
                nc.vector.tensor_add(outt[:], pp[:], x_sb[b][ct][:])
                nc.sync.dma_start(y_d[b, ct * 128:(ct + 1) * 128, :], outt[:])


        PIPE = 3
        for i in range(PIPE):
            emit_S(i)
        for i, (u, m) in enumerate(seq):
            if i + PIPE < len(seq):
                emit_S(i + PIPE)
            b, p, nch = units[u]
            ns = slice(nch * 512, (nch + 1) * 512)
            h0, h1 = 2 * p, 2 * p + 1
            if m == 0:
                po_tiles[u] = (
                    ps_o.tile([128, 512], F32, tag="o", name="po0"),
                    ps_o.tile([128, 512], F32, tag="o", name="po1"),
                )
            po0, po1 = po_tiles[u]
            ps = s_tiles.pop(i)
            ex = expool.tile([128, N], BF16, tag="ex", name="ex")
            nc.scalar.activation(ex[:], ps[:], Exp)
            first, last = (m == 0), (m == 7)
            # AV+Z: [V_h0|1] -> O rows 0:64, Zrep rows 64:128
            nc.tensor.matmul(
                po0[:], lhsT=_R(vt_sb[b][m][:, 128 * h0:128 * h0 + 128]),
                rhs=_R(ex[:, 0:512]), start=first, stop=last)
            # [1|V_h1] -> Zrep rows 0:64, O rows 64:128
            nc.tensor.matmul(
                po1[:], lhsT=_R(vt_sb[b][m][:, 128 * h1:128 * h1 + 128]),
                rhs=_R(ex[:, 512:1024]), start=first, stop=last)
            if not last:
                continue
            ot = o_sb[b][p]
            # Evacuate each po bank with one full-tile copy so the PSUM slot
            # frees immediately; normalize off-PSUM afterwards.
            poc0 = rzpool.tile([128, 512], F32, tag="poc0")
            nc.vector.tensor_copy(poc0[:], po0[:])
            poc1 = rzpool.tile([128, 512], F32, tag="poc1")
            nc.vector.tensor_copy(poc1[:], po1[:])
            # head h0: O at rows 0:64; Z replicated at rows 64:128
            zs0 = rzpool.tile([64, 512], F32, tag="zs0")
            nc.sync.dma_start(zs0[:], poc0[64:128, :])
            rzs0 = rzpool.tile([64, 512], F32, tag="rzs0")
            nc.vector.reciprocal_approx_fast(rzs0[:], zs0[:])
            nc.vector.tensor_mul(ot[0:64, ns], poc0[0:64, :], rzs0[:])
            # head h1: O at rows 64:128, Z at rows 0:64 -> recip then shift up
            rz1 = rzpool.tile([128, 512], F32, tag="rz", name="rz1")
            nc.vector.reciprocal_approx_fast(rz1[0:64, :], poc1[0:64, :])
            rzs1 = rzpool.tile([128, 512], F32, tag="rzs1")
            nc.sync.dma_start(rzs1[64:128, :], rz1[0:64, :])
            nc.vector.tensor_mul(ot[64:128, ns], poc1[64:128, :], rzs1[64:128, :])
            if (u + 1) % 4 == 0:
                emit_proj(b)
                if u == 3:
                    emit_vgemm(1)

    nc.compile()
    return nc


def prep_inputs(x, gn_gamma, gn_beta, qkv_w, qkv_b, proj_w, proj_b):
    """Host-side weight prep shared by kernel() and the CoreSim test."""
    x = np.ascontiguousarray(np.asarray(x, np.float32)).reshape(B, C, N)
    gn_gamma = np.asarray(gn_gamma, np.float32)
    gn_beta = np.asarray(gn_beta, np.float32)
    qkv_w = np.asarray(qkv_w, np.float32)
    qkv_b = np.asarray(qkv_b, np.float32)
    proj_w = np.asarray(proj_w, np.float32)
    proj_b = np.asarray(proj_b, np.float32)

    # fold GroupNorm affine into the qkv GEMM
    W3 = qkv_w * gn_gamma[None, :]
    b3 = qkv_b + qkv_w @ gn_beta
    W3r = W3.reshape(NH, 3, D, C)
    b3r = b3.reshape(NH, 3, D)
    scale = np.float32(D ** -0.5)
    Wq = W3r[:, 0].reshape(C, C)
    Wk = W3r[:, 1].reshape(C, C) * scale   # fold the attention scale into K
    Wv = W3r[:, 2].reshape(C, C)
    bq = b3r[:, 0].reshape(C)
    bk = b3r[:, 1].reshape(C) * scale
    bv = b3r[:, 2].reshape(C)

    wqk_t = np.ascontiguousarray(
        np.concatenate([Wq, Wk], axis=0).T).reshape(2, 128, 512)
    wv_t = np.ascontiguousarray(Wv.T).reshape(2, 128, 256)
    wp_t = np.ascontiguousarray(proj_w.T).reshape(2, 128, 256)
    bqk = np.concatenate([bq, bk]).reshape(4, 128)

    cidx = np.arange(128)
    gmap = np.zeros((128, 16), np.float32)
    gmap[cidx, cidx // 8] = 1.0 / 8.0
    gexp = np.zeros((16, 128), np.float32)
    gexp[cidx // 8, cidx] = 1.0

    common = {
        "wqk_t": wqk_t.astype(np.float32),
        "wv_t": wv_t.astype(np.float32),
        "wp_t": wp_t.astype(np.float32),
        "bqk": bqk.astype(np.float32),
        "bv": np.ascontiguousarray(bv[None, :], np.float32),
        "bp": np.ascontiguousarray(proj_b[None, :], np.float32),
        "gmap": gmap,
        "gexp": gexp,
    }
    in_maps = [
        {**common, "x": np.ascontiguousarray(x[c * BL:(c + 1) * BL])}
        for c in range(NCORES)
    ]
    return in_maps


_NC_CACHE = []


def kernel(x, gn_gamma, gn_beta, qkv_w, qkv_b, proj_w, proj_b, trace=False):
    in_maps = prep_inputs(x, gn_gamma, gn_beta, qkv_w, qkv_b, proj_w, proj_b)
    if not _NC_CACHE:
        _NC_CACHE.append(build_bass())
    nc = _NC_CACHE[0]
    res = run_bass_kernel_spmd(nc, in_maps, list(range(NCORES)), trace=trace)
    y = np.stack([res.results[c]["y"] for c in range(NCORES)])
    y = y.reshape(B, C, HH, WW)
    kernel.last_result = res
    return y


# revision 28
# speedup vs baseline: 1.0178x; 1.0178x over previous
"""AttentionBlock (GroupNorm + 4-head self-attention + proj + residual) on 8 trn2 cores.

Sharding: data-parallel over batch (B=16 -> 2 per core). Each core runs the full
block on its 2 batch elements; no collectives.

Device pipeline per batch (all layouts chosen so no on-device transposes are needed):
  - GroupNorm stats via bn_stats/bn_aggr + tiny PE matmuls for the cross-partition
    group combine (gamma/beta folded into the qkv weights on host).
  - Q,K GEMM in [channel, spatial] layout; V^T via a swapped GEMM (stationary=h).
  - Scores computed transposed: S^T[m,n] = K^T Q, so softmax's sum runs over the
    PSUM partition axis, computed by an all-ones stationary matmul that also
    replicates the denominator Z across partitions (no broadcast needed).
  - exp on ACT straight out of PSUM; no max-subtraction (scores bounded ~|8|).
  - AV as O^T[d,n] = sum_m V^T[m,d] expS^T[m,n], 2 heads packed per 128-col array.
  - proj GEMM + residual; biases injected via rank-1 (K=1) matmuls into PSUM.
"""

import numpy as np
from contextlib import ExitStack

import concourse.bass as bass
import concourse.bacc as bacc
import concourse.tile as tile
import concourse.mybir as mybir
from concourse.bass_utils import run_bass_kernel_spmd

F32 = mybir.dt.float32
F32R = mybir.dt.float32r
BF16 = mybir.dt.bfloat16

B, C, HH, WW = 16, 256, 32, 32
N = HH * WW           # 1024 spatial positions
NH = 4                # heads
D = C // NH           # 64 head dim
G = 32                # groups
EPS = 1e-5
NCORES = 8
BL = B // NCORES      # batches per core

USE_F32R = True       # fp32r matmuls: full PE rate; flip to False for exact fp32
MM_DT = F32R if USE_F32R else F32


def _R(ap):
    return ap


def build_bass():
    nc = bacc.Bacc("TRN2", target_bir_lowering=False, debug=False)

    x_d = nc.dram_tensor("x", [BL, C, N], F32, kind="ExternalInput").ap()
    wqk_d = nc.dram_tensor("wqk_t", [2, 128, 512], MM_DT, kind="ExternalInput").ap()
    wv_d = nc.dram_tensor("wv_t", [2, 128, 256], MM_DT, kind="ExternalInput").ap()
    wp_d = nc.dram_tensor("wp_t", [2, 128, 256], MM_DT, kind="ExternalInput").ap()
    bqk_d = nc.dram_tensor("bqk", [4, 128], F32, kind="ExternalInput").ap()
    bv_d = nc.dram_tensor("bv", [1, 256], MM_DT, kind="ExternalInput").ap()
    bp_d = nc.dram_tensor("bp", [1, 256], MM_DT, kind="ExternalInput").ap()
    gmap_d = nc.dram_tensor("gmap", [128, 16], F32, kind="ExternalInput").ap()
    gexp_d = nc.dram_tensor("gexp", [16, 128], F32, kind="ExternalInput").ap()
    y_d = nc.dram_tensor("y", [BL, C, N], F32, kind="ExternalOutput").ap()

    Exp = mybir.ActivationFunctionType.Exp
    mult = mybir.AluOpType.mult
    sub = mybir.AluOpType.subtract
    add = mybir.AluOpType.add

    with tile.TileContext(nc) as tc, ExitStack() as ctx:
        consts = ctx.enter_context(tc.tile_pool(name="consts", bufs=1))
        xpool = ctx.enter_context(tc.tile_pool(name="xp", bufs=1))
        hpool = ctx.enter_context(tc.tile_pool(name="hp", bufs=1))
        qkpool = ctx.enter_context(tc.tile_pool(name="qkp", bufs=1))
        vtpool = ctx.enter_context(tc.tile_pool(name="vtp", bufs=1))
        opool = ctx.enter_context(tc.tile_pool(name="op", bufs=1))
        gnpool = ctx.enter_context(tc.tile_pool(name="gnp", bufs=2))
        expool = ctx.enter_context(tc.tile_pool(name="exp", bufs=4))
        rzpool = ctx.enter_context(tc.tile_pool(name="rzp", bufs=3))
        outpool = ctx.enter_context(tc.tile_pool(name="outp", bufs=2))
        ps_big = ctx.enter_context(tc.tile_pool(name="psb", bufs=3, space="PSUM"))
        ps_o = ctx.enter_context(tc.tile_pool(name="pso", bufs=2, space="PSUM"))
        
        x_sb = [[None] * 2 for _ in range(BL)]
        h_sb = [[None] * 2 for _ in range(BL)]
        qk_sb = [[None] * 4 for _ in range(BL)]
        vt_sb = [[None] * 8 for _ in range(BL)]
        o_sb = [[None] * 2 for _ in range(BL)]

        # --- x first (GroupNorm gates everything), split for earlier stats ---
        for b in range(BL):
            for ct in range(2):
                xt = xpool.tile([128, N], F32, tag=f"x{b}{ct}", name=f"x{b}{ct}")
                x_sb[b][ct] = xt
                nc.sync.dma_start(xt[:, 0:512], x_d[b, ct * 128:(ct + 1) * 128, 0:512])
                nc.sync.dma_start(xt[:, 512:1024], x_d[b, ct * 128:(ct + 1) * 128, 512:1024])

        # --- constants / weights into SBUF ---
        wqk_sb = [consts.tile([128, 512], MM_DT, tag=f"wqk{k}", name=f"wqk{k}") for k in range(2)]
        wv_sb = [consts.tile([128, 256], MM_DT, tag=f"wv{k}", name=f"wv{k}") for k in range(2)]
        wp_sb = [consts.tile([128, 256], MM_DT, tag=f"wp{k}", name=f"wp{k}") for k in range(2)]
        gmap_sb = consts.tile([128, 16], F32, tag="gmap")
        nc.sync.dma_start(gmap_sb[:], gmap_d[:])
        gexp_sb = consts.tile([16, 128], F32, tag="gexp")
        nc.sync.dma_start(gexp_sb[:], gexp_d[:])
        bqk_sb = consts.tile([128, 4], F32, tag="bqk")
        nc.sync.dma_start(bqk_sb[:], bqk_d.transpose([1, 0]))
        for k in range(2):
            nc.sync.dma_start(wqk_sb[k][:], wqk_d[k])
            nc.sync.dma_start(wv_sb[k][:], wv_d[k])
            nc.sync.dma_start(wp_sb[k][:], wp_d[k])
        bv_sb = consts.tile([1, 256], MM_DT, tag="bv")
        nc.sync.dma_start(bv_sb[:], bv_d[:])
        bp_sb = consts.tile([1, 256], MM_DT, tag="bp")
        nc.sync.dma_start(bp_sb[:], bp_d[:])
        ones_f32 = consts.tile([128, 512], F32, tag="ones_f32")
        nc.vector.memset(ones_f32[:], 1.0)
        ones_sb = consts.tile([128, 512], MM_DT, tag="ones")
        nc.vector.tensor_copy(ones_sb[:], ones_f32[:])
        # bv broadcast to all partitions via a rank-1 matmul (done once)
        pbv = ps_o.tile([128, 256], F32, tag="o", name="pbv")
        nc.tensor.matmul(pbv[:], lhsT=_R(ones_sb[0:1, 0:128]), rhs=_R(bv_sb[0:1, :]),
                         start=True, stop=True)
        bvb = consts.tile([128, 256], F32, tag="bvb")
        nc.vector.tensor_copy(bvb[:], pbv[:])

        # ====== Phase A: GroupNorm, batched 4-wide across (b, ct) tiles ======
        # Stats per tile, then ONE short scalar-math chain on [128, 4] strided
        # views instead of four serial per-tile chains.
        units4 = [(b, ct) for b in range(BL) for ct in range(2)]
        bn6 = gnpool.tile([128, 48], F32, tag="bn6")
        mva = gnpool.tile([128, 8], F32, tag="mva")  # (mean, var) x 4 units
        for u4, (b, ct) in enumerate(units4):
            xt = x_sb[b][ct]
            nc.vector.bn_stats(bn6[:, 12 * u4:12 * u4 + 6], xt[:, 0:512])
            nc.vector.bn_stats(bn6[:, 12 * u4 + 6:12 * u4 + 12], xt[:, 512:1024])
            nc.vector.bn_aggr(mva[:, 2 * u4:2 * u4 + 2], bn6[:, 12 * u4:12 * u4 + 12])
        mva2 = mva[:].rearrange("p (u c) -> p u c", c=2)
        m2a = gnpool.tile([128, 4], F32, tag="m2a")
        nc.vector.tensor_mul(m2a[:], mva2[:, :, 0], mva2[:, :, 0])
        # group-combine via matmul: [sum mean, sum var]x4 then [sum mean^2]x4
        psg = ps_o.tile([16, 12], F32, tag="o", name="psg")
        nc.tensor.matmul(psg[:, 0:8], lhsT=gmap_sb[:], rhs=mva[:], start=True, stop=True,
                         skip_group_check=True)
        nc.tensor.matmul(psg[:, 8:12], lhsT=gmap_sb[:], rhs=m2a[:], start=True, stop=True,
                         skip_group_check=True)
        gsa = gnpool.tile([16, 12], F32, tag="gsa")
        nc.vector.tensor_copy(gsa[:], psg[:])
        gsa2 = gsa[:, 0:8].rearrange("p (u c) -> p u c", c=2)
        nc.vector.tensor_add(gsa2[:, :, 1], gsa2[:, :, 1], gsa[:, 8:12])
        psc = ps_o.tile([128, 8], F32, tag="o", name="psc")
        nc.tensor.matmul(psc[:], lhsT=gexp_sb[:], rhs=gsa[:, 0:8], start=True, stop=True)
        csa = gnpool.tile([128, 8], F32, tag="csa")  # [mean_g, E[x^2]_g] x 4
        nc.vector.tensor_copy(csa[:], psc[:])
        csa2 = csa[:].rearrange("p (u c) -> p u c", c=2)
        m2g = gnpool.tile([128, 4], F32, tag="m2g")
        nc.vector.tensor_mul(m2g[:], csa2[:, :, 0], csa2[:, :, 0])
        vea = gnpool.tile([128, 4], F32, tag="vea")
        nc.vector.tensor_sub(vea[:], csa2[:, :, 1], m2g[:])
        nc.vector.tensor_scalar_add(vea[:], vea[:], EPS)
        iva = gnpool.tile([128, 4], F32, tag="iva")
        nc.vector.reciprocal(iva[:], vea[:])
        s_a = gnpool.tile([128, 4], F32, tag="s_a")
        nc.scalar.sqrt(s_a[:], iva[:])
        t_a = gnpool.tile([128, 4], F32, tag="t_a")  # mean * inv_std
        nc.vector.tensor_mul(t_a[:], csa2[:, :, 0], s_a[:])
        # PE warm-up: HAM needs ~3.4us of sustained activity; these rewrite
        # pbv with the same value while the DVE finishes the GN chain.
        for w in range(14):
            nc.tensor.matmul(pbv[:], lhsT=_R(ones_sb[0:1, 0:128]),
                             rhs=_R(bv_sb[0:1, :]), start=True, stop=True)
        for u4, (b, ct) in enumerate(units4):
            # h = x * s - t  (gamma/beta already folded into W/b on host)
            ht = hpool.tile([128, N], MM_DT, tag=f"h{b}{ct}", name=f"h{b}{ct}")
            h_sb[b][ct] = ht
            nc.vector.tensor_scalar(ht[:], x_sb[b][ct][:], s_a[:, u4:u4 + 1],
                                    t_a[:, u4:u4 + 1], mult, sub)

        for b in range(BL):
            # Q,K GEMM: out channels ot: 0=q01 1=q23 2=k01 3=k23 (K pre-scaled by 1/8)
            for ot in range(4):
                pq = ps_big.tile([128, N], F32, tag="big")
                for nch in range(2):
                    ns = slice(nch * 512, (nch + 1) * 512)
                    for k in range(2):
                        nc.tensor.matmul(
                            pq[:, ns],
                            lhsT=_R(wqk_sb[k][:, ot * 128:(ot + 1) * 128]),
                            rhs=_R(h_sb[b][k][:, ns]),
                            start=(k == 0), stop=(k == 1),
                        )
                qk = qkpool.tile([128, N], MM_DT, tag=f"qk{b}{ot}")
                qk_sb[b][ot] = qk
                nc.vector.tensor_scalar(qk[:], pq[:], bqk_sb[:, ot:ot + 1], None, add)

            # V^T GEMM: V^T[m, vc] = sum_c h[c,m] WvT[c,vc]  (+ bv via rank-1 matmul)
            for m in range(8):
                pv = ps_o.tile([128, 512], F32, tag="o")
                mc = slice(m * 128, (m + 1) * 128)
                for k in range(2):
                    nc.tensor.matmul(
                        pv[:, 0:256],
                        lhsT=_R(h_sb[b][k][:, mc]),
                        rhs=_R(wv_sb[k][:]),
                        start=(k == 0), stop=(k == 1),
                    )
                # vt layout [128, 512]: per global head h a 128-col block;
                # even h: [V_h | ones], odd h: [ones | V_h].  The ones columns
                # make the AV matmul also emit the softmax denominator Z
                # (replicated over 64 partitions) in the same PSUM bank.
                vt = vtpool.tile([128, 512], BF16, tag=f"vt{b}{m}")
                vt_sb[b][m] = vt
                vt4 = vt[:].rearrange("p (a u v d) -> p a u v d", a=2, u=2, v=2)
                pv4 = pv[:, 0:256].rearrange("p (a w d) -> p a w d", a=2, w=2)
                of = ones_f32[:, 0:128].rearrange("p (a d) -> p a d", a=2)
                bvb4 = bvb[:].rearrange("p (a w d) -> p a w d", a=2, w=2)
                nc.vector.tensor_add(vt4[:, :, 0, 0, :], pv4[:, :, 0, :], bvb4[:, :, 0, :])
                nc.vector.tensor_add(vt4[:, :, 1, 1, :], pv4[:, :, 1, :], bvb4[:, :, 1, :])
                vtq = vt[:].rearrange("p (a q d) -> p a q d", a=2, q=4)
                nc.vector.tensor_copy(vtq[:, :, 1:3, :], ones_f32[:, 0:256].rearrange(
                    "p (a d) -> p a d", a=2).rearrange("p a (u d) -> p a u d", u=2))

        # ================= Phase B: attention, Phase C: proj+residual ======
        # Software-pipelined emission over all (b, p, nch) units x 8 m-chunks:
        # the PE executes its queue in order, so S^T matmuls are emitted two
        # steps ahead of the exp-dependent AV+Z matmuls to keep the PE from
        # stalling on ACT.
        for b in range(BL):
            for p in range(2):
                o_sb[b][p] = opool.tile([128, N], MM_DT, tag=f"o{b}{p}",
                                        name=f"ot{b}{p}")
        units = [(b, p, nch) for b in range(BL) for p in range(2) for nch in range(2)]
        seq = [(u, m) for u in range(len(units)) for m in range(8)]
        s_tiles = {}
        po_tiles = {}

        def emit_S(i):
            u, m = seq[i]
            b, p, nch = units[u]
            qt, kt = qk_sb[b][p], qk_sb[b][2 + p]
            ns = slice(nch * 512, (nch + 1) * 512)
            mc = slice(m * 128, (m + 1) * 128)
            ps = ps_big.tile([128, N], F32, tag="big", name="ps")
            nc.tensor.matmul(ps[:, 0:512], lhsT=_R(kt[0:64, mc]),
                             rhs=_R(qt[0:64, ns]), start=True, stop=True)
            nc.tensor.matmul(ps[:, 512:1024], lhsT=_R(kt[64:128, mc]),
                             rhs=_R(qt[64:128, ns]), start=True, stop=True)
            s_tiles[i] = ps

        def emit_proj(b):
            # proj + residual + bias
            for ct in range(2):
                pp = ps_big.tile([128, N], F32, tag="big", name="pp")
                for nch in range(2):
                    ns = slice(nch * 512, (nch + 1) * 512)
                    for k in range(2):
                        nc.tensor.matmul(
                            pp[:, ns],
                            lhsT=_R(wp_sb[k][:, ct * 128:(ct + 1) * 128]),
                            rhs=_R(o_sb[b][k][:, ns]),
                            start=(k == 0), stop=False,
                        )
                    nc.tensor.matmul(
                        pp[:, ns],
                        lhsT=_R(bp_sb[0:1, ct * 128:(ct + 1) * 128]),
                        rhs=_R(ones_sb[0:1, :]),
                        start=False, stop=True,
                    )
                outt = outpool.tile([128, N], F32, tag="out")
                nc.vector.tensor_add(outt[:], pp[:], x_sb[b][ct][:])
                nc.sync.dma_start(y_d[b, ct * 128:(ct + 1) * 128, :], outt[:])


        PIPE = 3
        for i in range(PIPE):
            emit_S(i)
        for i, (u, m) in enumerate(seq):
            if i + PIPE < len(seq):
                emit_S(i + PIPE)
            b, p, nch = units[u]
            ns = slice(nch * 512, (nch + 1) * 512)
            h0, h1 = 2 * p, 2 * p + 1
            if m == 0:
                po_tiles[u] = (
                    ps_o.tile([128, 512], F32, tag="o", name="po0"),
                    ps_o.tile([128, 512], F32, tag="o", name="po1"),
                )
            po0, po1 = po_tiles[u]
            ps = s_tiles.pop(i)
            ex = expool.tile([128, N], BF16, tag="ex", name="ex")
            nc.scalar.activation(ex[:], ps[:], Exp)
            first, last = (m == 0), (m == 7)
            # AV+Z: [V_h0|1] -> O rows 0:64, Zrep rows 64:128
            nc.tensor.matmul(
                po0[:], lhsT=_R(vt_sb[b][m][:, 128 * h0:128 * h0 + 128]),
                rhs=_R(ex[:, 0:512]), start=first, stop=last)
            # [1|V_h1] -> Zrep rows 0:64, O rows 64:128
            nc.tensor.matmul(
                po1[:], lhsT=_R(vt_sb[b][m][:, 128 * h1:128 * h1 + 128]),
                rhs=_R(ex[:, 512:1024]), start=first, stop=last)
            if not last:
                continue
            ot = o_sb[b][p]
            # Evacuate each po bank with one full-tile copy so the PSUM slot
            # frees immediately; normalize off-PSUM afterwards.
            poc0 = rzpool.tile([128, 512], F32, tag="poc0")
            nc.vector.tensor_copy(poc0[:], po0[:])
            poc1 = rzpool.tile([128, 512], F32, tag="poc1")
            nc.vector.tensor_copy(poc1[:], po1[:])
            # head h0: O at rows 0:64; Z replicated at rows 64:128
            zs0 = rzpool.tile([64, 512], F32, tag="zs0")
            nc.sync.dma_start(zs0[:], poc0[64:128, :])
            rzs0 = rzpool.tile([64, 512], F32, tag="rzs0")
            nc.vector.reciprocal_approx_fast(rzs0[:], zs0[:])
            nc.vector.tensor_mul(ot[0:64, ns], poc0[0:64, :], rzs0[:])
            # head h1: O at rows 64:128, Z at rows 0:64 -> recip then shift up
            rz1 = rzpool.tile([128, 512], F32, tag="rz", name="rz1")
            nc.vector.reciprocal_approx_fast(rz1[0:64, :], poc1[0:64, :])
            rzs1 = rzpool.tile([128, 512], F32, tag="rzs1")
            nc.sync.dma_start(rzs1[64:128, :], rz1[0:64, :])
            nc.vector.tensor_mul(ot[64:128, ns], poc1[64:128, :], rzs1[64:128, :])
            if (u + 1) % 4 == 0:
                emit_proj(b)

    nc.compile()
    return nc


def prep_inputs(x, gn_gamma, gn_beta, qkv_w, qkv_b, proj_w, proj_b):
    """Host-side weight prep shared by kernel() and the CoreSim test."""
    x = np.ascontiguousarray(np.asarray(x, np.float32)).reshape(B, C, N)
    gn_gamma = np.asarray(gn_gamma, np.float32)
    gn_beta = np.asarray(gn_beta, np.float32)
    qkv_w = np.asarray(qkv_w, np.float32)
    qkv_b = np.asarray(qkv_b, np.float32)
    proj_w = np.asarray(proj_w, np.float32)
    proj_b = np.asarray(proj_b, np.float32)

    # fold GroupNorm affine into the qkv GEMM
    W3 = qkv_w * gn_gamma[None, :]
    b3 = qkv_b + qkv_w @ gn_beta
    W3r = W3.reshape(NH, 3, D, C)
    b3r = b3.reshape(NH, 3, D)
    scale = np.float32(D ** -0.5)
    Wq = W3r[:, 0].reshape(C, C)
    Wk = W3r[:, 1].reshape(C, C) * scale   # fold the attention scale into K
    Wv = W3r[:, 2].reshape(C, C)
    bq = b3r[:, 0].reshape(C)
    bk = b3r[:, 1].reshape(C) * scale
    bv = b3r[:, 2].reshape(C)

    wqk_t = np.ascontiguousarray(
        np.concatenate([Wq, Wk], axis=0).T).reshape(2, 128, 512)
    wv_t = np.ascontiguousarray(Wv.T).reshape(2, 128, 256)
    wp_t = np.ascontiguousarray(proj_w.T).reshape(2, 128, 256)
    bqk = np.concatenate([bq, bk]).reshape(4, 128)

    cidx = np.arange(128)
    gmap = np.zeros((128, 16), np.float32)
    gmap[cidx, cidx // 8] = 1.0 / 8.0
    gexp = np.zeros((16, 128), np.float32)
    gexp[cidx // 8, cidx] = 1.0

    common = {
        "wqk_t": wqk_t.astype(np.float32),
        "wv_t": wv_t.astype(np.float32),
        "wp_t": wp_t.astype(np.float32),
        "bqk": bqk.astype(np.float32),
        "bv": np.ascontiguousarray(bv[None, :], np.float32),
        "bp": np.ascontiguousarray(proj_b[None, :], np.float32),
        "gmap": gmap,
        "gexp": gexp,
    }
    in_maps = [
        {**common, "x": np.ascontiguousarray(x[c * BL:(c + 1) * BL])}
        for c in range(NCORES)
    ]
    return in_maps


_NC_CACHE = []


def kernel(x, gn_gamma, gn_beta, qkv_w, qkv_b, proj_w, proj_b, trace=False):
    in_maps = prep_inputs(x, gn_gamma, gn_beta, qkv_w, qkv_b, proj_w, proj_b)
    if not _NC_CACHE:
        _NC_CACHE.append(build_bass())
    nc = _NC_CACHE[0]
    res = run_bass_kernel_spmd(nc, in_maps, list(range(NCORES)), trace=trace)
    y = np.stack([res.results[c]["y"] for c in range(NCORES)])
    y = y.reshape(B, C, HH, WW)
    kernel.last_result = res
    return y
